# revision 7
# baseline (speedup 1.0000x reference)
"""Trainium2 Bass kernel: out = softmax(gelu_tanh(x @ W^T), axis=-1) + bias.

Full shapes: x [8192, 4096] f32, weight [4096, 4096] f32, bias [4096] f32.
Sharding: data-parallel over rows of x across 8 NeuronCores (1024 rows/core);
weight and bias replicated. Matmul runs in bf16 on the PE array with fp32
PSUM accumulation; gelu is computed with the exact tanh-approx constants of
the reference via DVE + ACT(Tanh), and softmax needs no max-subtraction
because gelu output is bounded in [-0.17, ~3.5] so exp cannot overflow.

Per-core loop structure (MC=1024 rows):
  split rows into G=2 groups of 512; for each group, stream weight n-tiles
  (512 cols) with the x-group resident in SBUF; accumulate 32 k-matmuls into
  PSUM per (m-tile, n-tile); fuse exp(gelu(v)) into the PSUM->SBUF epilogue
  with per-row sums accumulated by the ACT engine; normalize + bias-add with
  one fused DVE op per tile, then DMA out.
"""

import sys

if "/opt/trn_rl_repo" not in sys.path:
    sys.path.insert(0, "/opt/trn_rl_repo")

import ml_dtypes
import numpy as np

import concourse.bass as bass
import concourse.tile as tile
from concourse import bacc, mybir
from concourse.bass_utils import run_bass_kernel_spmd

P = 128
GELU_A = 0.044715
GELU_C = 0.7978845608

# Full-problem constants (hardcoded; harness calls kernel() with these shapes)
FULL_M, FULL_K, FULL_N = 8192, 4096, 4096
NCORES = 8
MC = FULL_M // NCORES  # rows per core
G = 2                  # row groups per core
NT = 512               # n tile (columns per weight tile / psum)


def build_nc(MC=MC, K=FULL_K, N=FULL_N, G=G, NT=NT):
    """Emit the per-core Bass program. Each core computes MC rows."""
    KO = K // P            # k subtiles of 128
    MG = MC // G           # rows per m-group
    MT = MG // P           # 128-row m-tiles per group
    NTILES = N // NT
    f32 = mybir.dt.float32
    bf16 = mybir.dt.bfloat16

    nc = bacc.Bacc("TRN2", target_bir_lowering=False, debug=False)
    xt = nc.dram_tensor("xt", [G, P, KO, MG], bf16, kind="ExternalInput").ap()
    wt = nc.dram_tensor("wt", [NTILES, P, KO, NT], bf16, kind="ExternalInput").ap()
    bias = nc.dram_tensor("bias", [P, N], f32, kind="ExternalInput").ap()
    out = nc.dram_tensor("out", [P, MC // P, N], f32, kind="ExternalOutput").ap()

    with tile.TileContext(nc) as tc:
        with (
            tc.tile_pool(name="const", bufs=1) as const_pool,
            tc.tile_pool(name="x", bufs=1) as x_pool,
            tc.tile_pool(name="w", bufs=2) as w_pool,
            tc.tile_pool(name="probs", bufs=1) as probs_pool,
            tc.tile_pool(name="tmp", bufs=2) as tmp_pool,
            tc.tile_pool(name="stat", bufs=2) as stat_pool,
            tc.tile_pool(name="stage", bufs=4) as stage_pool,
            tc.tile_pool(name="psum", bufs=6, space="PSUM") as psum_pool,
        ):
            bias_t = const_pool.tile([P, N], f32)
            nc.gpsimd.dma_start(bias_t[:], bias[:])

            for g in range(G):
                x_t = x_pool.tile([P, KO, MG], bf16)
                nc.gpsimd.dma_start(x_t[:], xt[g])
                probs = probs_pool.tile([P, MT, N], bf16)
                sums = stat_pool.tile([P, MT * NTILES], f32, tag="sums")
                for j in range(NTILES):
                    w_t = w_pool.tile([P, KO, NT], bf16)
                    nc.gpsimd.dma_start(w_t[:], wt[j])
                    for i in range(MT):
                        ps = psum_pool.tile([P, NT], f32)
                        for k in range(KO):
                            nc.tensor.matmul(
                                ps[:],
                                x_t[:, k, i * P : (i + 1) * P],
                                w_t[:, k, :],
                                start=(k == 0),
                                stop=(k == KO - 1),
                            )
                        # p = exp(gelu(v)) with gelu = 0.5*v*(1+tanh(C*(v+A*v^3)))
                        # (HW: only one PSUM input per instruction -> copy out once)
                        vs = tmp_pool.tile([P, NT], f32, tag="vs")
                        nc.scalar.copy(vs[:], ps[:])
                        v2 = tmp_pool.tile([P, NT], f32, tag="v2")
                        nc.vector.tensor_mul(v2[:], vs[:], vs[:])
                        t1 = tmp_pool.tile([P, NT], f32, tag="t1")
                        nc.vector.tensor_scalar(
                            t1[:], v2[:], GELU_A, 1.0,
                            mybir.AluOpType.mult, mybir.AluOpType.add,
                        )
                        t2 = tmp_pool.tile([P, NT], f32, tag="t2")
                        nc.vector.tensor_mul(t2[:], vs[:], t1[:])
                        th = tmp_pool.tile([P, NT], f32, tag="th")
                        nc.scalar.activation(
                            th[:], t2[:], mybir.ActivationFunctionType.Tanh,
                            bias=0.0, scale=GELU_C,
                        )
                        g2 = tmp_pool.tile([P, NT], f32, tag="g2")
                        nc.vector.scalar_tensor_tensor(
                            g2[:], th[:], 1.0, vs[:],
                            mybir.AluOpType.add, mybir.AluOpType.mult,
                        )
                        sidx = i * NTILES + j
                        nc.scalar.activation(
                            probs[:, i, j * NT : (j + 1) * NT], g2[:],
                            mybir.ActivationFunctionType.Exp,
                            bias=0.0, scale=0.5,
                            accum_out=sums[:, sidx : sidx + 1],
                        )
                ssum = stat_pool.tile([P, MT], f32, tag="ssum")
                recips = stat_pool.tile([P, MT], f32, tag="recips")
                for i in range(MT):
                    nc.vector.reduce_sum(
                        ssum[:, i : i + 1],
                        sums[:, i * NTILES : (i + 1) * NTILES],
                        axis=mybir.AxisListType.X,
                    )
                    nc.vector.reciprocal(recips[:, i : i + 1], ssum[:, i : i + 1])
                    for j in range(NTILES):
                        st = stage_pool.tile([P, NT], f32)
                        nc.vector.scalar_tensor_tensor(
                            st[:],
                            probs[:, i, j * NT : (j + 1) * NT],
                            recips[:, i : i + 1],
                            bias_t[:, j * NT : (j + 1) * NT],
                            mybir.AluOpType.mult,
                            mybir.AluOpType.add,
                        )
                        nc.gpsimd.dma_start(out[:, g * MT + i, j * NT : (j + 1) * NT], st[:])
    nc.compile()
    return nc


def pack_inputs(x, weight, bias, MC=MC, G=G, NT=NT):
    """Host-side shard + pack into the DMA-friendly layouts the kernel expects."""
    M, K = x.shape
    N = weight.shape[0]
    KO = K // P
    MG = MC // G
    NTILES = N // NT
    ncores = M // MC
    bf = ml_dtypes.bfloat16
    # wt[j, p, ko, n] = weight[j*NT+n, ko*P+p]
    wt = np.ascontiguousarray(
        weight.astype(bf).reshape(NTILES, NT, KO, P).transpose(0, 3, 2, 1)
    )
    bias_b = np.ascontiguousarray(
        np.broadcast_to(bias.astype(np.float32)[None, :], (P, N))
    )
    in_maps = []
    for c in range(ncores):
        xs = x[c * MC : (c + 1) * MC].astype(bf)
        # xt[g, p, ko, m] = x_core[g*MG+m, ko*P+p]
        xtc = np.ascontiguousarray(xs.reshape(G, MG, KO, P).transpose(0, 3, 2, 1))
        in_maps.append({"xt": xtc, "wt": wt, "bias": bias_b})
    return in_maps


def unpack_outputs(results, MC=MC, N=FULL_N):
    outs = []
    for res in results:
        o = np.asarray(res["out"])  # [P, MC//P, N]
        outs.append(o.transpose(1, 0, 2).reshape(MC, N))
    return np.concatenate(outs, axis=0)


_CACHE = {}


def _get_nc():
    if "nc" not in _CACHE:
        _CACHE["nc"] = build_nc()
    return _CACHE["nc"]


def _ensure_trace_env():
    """The agent image's antenv lacks axon_hooks, so NTFF tracing silently
    degrades. Register the ctypes-based hook ourselves, and neuter the S3
    artifact upload (no bucket access here)."""
    try:
        from antenv.axon_hooks import get_axon_ntff_profile_hook  # noqa: F401
    except ImportError:
        import types

        import antenv
        from trn_agent_boot.trn_boot import _ntff_profile_via_ctypes

        mod = types.ModuleType("antenv.axon_hooks")
        state = {"hook": _ntff_profile_via_ctypes("/opt/axon/libaxon_pjrt.so")}
        mod.set_axon_ntff_profile_hook = lambda h: state.__setitem__("hook", h)
        mod.get_axon_ntff_profile_hook = lambda: state["hook"]
        sys.modules["antenv.axon_hooks"] = mod
        antenv.axon_hooks = mod
    import concourse.bass_utils as bu

    bu.upload_artifacts = lambda tmpdir: f"local://{tmpdir}"


def kernel(x, weight, bias, trace=False):
    if trace:
        _ensure_trace_env()
    nc = _get_nc()
    in_maps = pack_inputs(
        np.asarray(x, dtype=np.float32),
        np.asarray(weight, dtype=np.float32),
        np.asarray(bias, dtype=np.float32),
    )
    res = run_bass_kernel_spmd(nc, in_maps, core_ids=list(range(NCORES)), trace=trace)
    out = unpack_outputs(res.results)
    if trace:
        return out, res
    return out


# revision 9
# speedup vs baseline: 1.0386x; 1.0386x over previous
"""Trainium2 Bass kernel: out = softmax(gelu_tanh(x @ W^T), axis=-1) + bias.

Full shapes: x [8192, 4096] f32, weight [4096, 4096] f32, bias [4096] f32.
Sharding: data-parallel over rows of x across 8 NeuronCores (1024 rows/core);
weight and bias replicated. Matmul runs in bf16 on the PE array with fp32
PSUM accumulation; gelu is computed with the exact tanh-approx constants of
the reference via DVE + ACT(Tanh), and softmax needs no max-subtraction
because gelu output is bounded in [-0.17, ~3.5] so exp cannot overflow.

Per-core loop structure (MC=1024 rows):
  split rows into G=2 groups of 512; for each group, stream weight n-tiles
  (512 cols) with the x-group resident in SBUF; accumulate 32 k-matmuls into
  PSUM per (m-tile, n-tile); fuse exp(gelu(v)) into the PSUM->SBUF epilogue
  with per-row sums accumulated by the ACT engine; normalize + bias-add with
  one fused DVE op per tile, then DMA out.
"""

import sys

if "/opt/trn_rl_repo" not in sys.path:
    sys.path.insert(0, "/opt/trn_rl_repo")

import ml_dtypes
import numpy as np

import concourse.bass as bass
import concourse.tile as tile
from concourse import bacc, mybir
from concourse.bass_utils import run_bass_kernel_spmd

P = 128
GELU_A = 0.044715
GELU_C = 0.7978845608

# Full-problem constants (hardcoded; harness calls kernel() with these shapes)
FULL_M, FULL_K, FULL_N = 8192, 4096, 4096
NCORES = 8
MC = FULL_M // NCORES  # rows per core
G = 2                  # row groups per core
NT = 512               # n tile (columns per weight tile / psum)


def build_nc(MC=MC, K=FULL_K, N=FULL_N, G=G, NT=NT):
    """Emit the per-core Bass program. Each core computes MC rows."""
    KO = K // P            # k subtiles of 128
    MG = MC // G           # rows per m-group
    MT = MG // P           # 128-row m-tiles per group
    NTILES = N // NT
    f32 = mybir.dt.float32
    bf16 = mybir.dt.bfloat16

    nc = bacc.Bacc("TRN2", target_bir_lowering=False, debug=False)
    xt = nc.dram_tensor("xt", [G, P, KO, MG], bf16, kind="ExternalInput").ap()
    wt = nc.dram_tensor("wt", [NTILES, P, KO, NT], bf16, kind="ExternalInput").ap()
    bias = nc.dram_tensor("bias", [P, N], f32, kind="ExternalInput").ap()
    out = nc.dram_tensor("out", [P, MC // P, N], f32, kind="ExternalOutput").ap()

    with tile.TileContext(nc) as tc:
        # k-chunking of the streaming DMAs: matmuls can start as soon as the
        # first chunk lands (Tile tracks slice-level deps), instead of waiting
        # for a full 4MB tile. x gets one spare slot so the next group's first
        # chunk prefetches while the current group is still computing.
        XCH = 4          # x chunks per group
        KX = KO // XCH
        WCH = 4          # w chunks per n-tile
        KW = KO // WCH
        with (
            tc.tile_pool(name="const", bufs=1) as const_pool,
            tc.tile_pool(name="x", bufs=XCH + 1) as x_pool,
            tc.tile_pool(name="w", bufs=2) as w_pool,
            tc.tile_pool(name="probs", bufs=1) as probs_pool,
            tc.tile_pool(name="tmp", bufs=2) as tmp_pool,
            tc.tile_pool(name="stat", bufs=2) as stat_pool,
            tc.tile_pool(name="stage", bufs=4) as stage_pool,
            tc.tile_pool(name="psum", bufs=6, space="PSUM") as psum_pool,
        ):
            bias_t = const_pool.tile([P, N], f32)
            nc.gpsimd.dma_start(bias_t[:], bias[:])

            for g in range(G):
                xcs = []
                for c in range(XCH):
                    xc = x_pool.tile([P, KX, MG], bf16, tag="xc")
                    nc.gpsimd.dma_start(xc[:], xt[g, :, c * KX : (c + 1) * KX, :])
                    xcs.append(xc)
                probs = probs_pool.tile([P, MT, N], bf16)
                sums = stat_pool.tile([P, MT * NTILES], f32, tag="sums")
                for j in range(NTILES):
                    w_t = w_pool.tile([P, KO, NT], bf16)
                    for c in range(WCH):
                        nc.gpsimd.dma_start(
                            w_t[:, c * KW : (c + 1) * KW, :],
                            wt[j, :, c * KW : (c + 1) * KW, :],
                        )
                    for i in range(MT):
                        ps = psum_pool.tile([P, NT], f32)
                        for k in range(KO):
                            nc.tensor.matmul(
                                ps[:],
                                xcs[k // KX][:, k % KX, i * P : (i + 1) * P],
                                w_t[:, k, :],
                                start=(k == 0),
                                stop=(k == KO - 1),
                            )
                        # p = exp(gelu(v)) with gelu = 0.5*v*(1+tanh(C*(v+A*v^3)))
                        # (HW: only one PSUM input per instruction -> copy out once)
                        vs = tmp_pool.tile([P, NT], f32, tag="vs")
                        nc.scalar.copy(vs[:], ps[:])
                        v2 = tmp_pool.tile([P, NT], f32, tag="v2")
                        nc.vector.tensor_mul(v2[:], vs[:], vs[:])
                        t1 = tmp_pool.tile([P, NT], f32, tag="t1")
                        nc.vector.tensor_scalar(
                            t1[:], v2[:], GELU_A, 1.0,
                            mybir.AluOpType.mult, mybir.AluOpType.add,
                        )
                        t2 = tmp_pool.tile([P, NT], f32, tag="t2")
                        nc.vector.tensor_mul(t2[:], vs[:], t1[:])
                        th = tmp_pool.tile([P, NT], f32, tag="th")
                        nc.scalar.activation(
                            th[:], t2[:], mybir.ActivationFunctionType.Tanh,
                            bias=0.0, scale=GELU_C,
                        )
                        g2 = tmp_pool.tile([P, NT], f32, tag="g2")
                        nc.vector.scalar_tensor_tensor(
                            g2[:], th[:], 1.0, vs[:],
                            mybir.AluOpType.add, mybir.AluOpType.mult,
                        )
                        sidx = i * NTILES + j
                        nc.scalar.activation(
                            probs[:, i, j * NT : (j + 1) * NT], g2[:],
                            mybir.ActivationFunctionType.Exp,
                            bias=0.0, scale=0.5,
                            accum_out=sums[:, sidx : sidx + 1],
                        )
                ssum = stat_pool.tile([P, MT], f32, tag="ssum")
                recips = stat_pool.tile([P, MT], f32, tag="recips")
                for i in range(MT):
                    nc.vector.reduce_sum(
                        ssum[:, i : i + 1],
                        sums[:, i * NTILES : (i + 1) * NTILES],
                        axis=mybir.AxisListType.X,
                    )
                    nc.vector.reciprocal(recips[:, i : i + 1], ssum[:, i : i + 1])
                    for j in range(NTILES):
                        st = stage_pool.tile([P, NT], f32)
                        nc.vector.scalar_tensor_tensor(
                            st[:],
                            probs[:, i, j * NT : (j + 1) * NT],
                            recips[:, i : i + 1],
                            bias_t[:, j * NT : (j + 1) * NT],
                            mybir.AluOpType.mult,
                            mybir.AluOpType.add,
                        )
                        nc.gpsimd.dma_start(out[:, g * MT + i, j * NT : (j + 1) * NT], st[:])
    nc.compile()
    return nc


def pack_inputs(x, weight, bias, MC=MC, G=G, NT=NT):
    """Host-side shard + pack into the DMA-friendly layouts the kernel expects."""
    M, K = x.shape
    N = weight.shape[0]
    KO = K // P
    MG = MC // G
    NTILES = N // NT
    ncores = M // MC
    bf = ml_dtypes.bfloat16
    # wt[j, p, ko, n] = weight[j*NT+n, ko*P+p]
    wt = np.ascontiguousarray(
        weight.astype(bf).reshape(NTILES, NT, KO, P).transpose(0, 3, 2, 1)
    )
    bias_b = np.ascontiguousarray(
        np.broadcast_to(bias.astype(np.float32)[None, :], (P, N))
    )
    in_maps = []
    for c in range(ncores):
        xs = x[c * MC : (c + 1) * MC].astype(bf)
        # xt[g, p, ko, m] = x_core[g*MG+m, ko*P+p]
        xtc = np.ascontiguousarray(xs.reshape(G, MG, KO, P).transpose(0, 3, 2, 1))
        in_maps.append({"xt": xtc, "wt": wt, "bias": bias_b})
    return in_maps


def unpack_outputs(results, MC=MC, N=FULL_N):
    outs = []
    for res in results:
        o = np.asarray(res["out"])  # [P, MC//P, N]
        outs.append(o.transpose(1, 0, 2).reshape(MC, N))
    return np.concatenate(outs, axis=0)


_CACHE = {}


def _get_nc():
    if "nc" not in _CACHE:
        _CACHE["nc"] = build_nc()
    return _CACHE["nc"]


def _ensure_trace_env():
    """The agent image's antenv lacks axon_hooks, so NTFF tracing silently
    degrades. Register the ctypes-based hook ourselves, and neuter the S3
    artifact upload (no bucket access here)."""
    try:
        from antenv.axon_hooks import get_axon_ntff_profile_hook  # noqa: F401
    except ImportError:
        import types

        import antenv
        from trn_agent_boot.trn_boot import _ntff_profile_via_ctypes

        mod = types.ModuleType("antenv.axon_hooks")
        state = {"hook": _ntff_profile_via_ctypes("/opt/axon/libaxon_pjrt.so")}
        mod.set_axon_ntff_profile_hook = lambda h: state.__setitem__("hook", h)
        mod.get_axon_ntff_profile_hook = lambda: state["hook"]
        sys.modules["antenv.axon_hooks"] = mod
        antenv.axon_hooks = mod
    import concourse.bass_utils as bu

    bu.upload_artifacts = lambda tmpdir: f"local://{tmpdir}"


def kernel(x, weight, bias, trace=False):
    if trace:
        _ensure_trace_env()
    nc = _get_nc()
    in_maps = pack_inputs(
        np.asarray(x, dtype=np.float32),
        np.asarray(weight, dtype=np.float32),
        np.asarray(bias, dtype=np.float32),
    )
    res = run_bass_kernel_spmd(nc, in_maps, core_ids=list(range(NCORES)), trace=trace)
    out = unpack_outputs(res.results)
    if trace:
        return out, res
    return out


# revision 16
# speedup vs baseline: 1.5735x; 1.5151x over previous
"""Trainium2 Bass kernel: out = softmax(gelu_tanh(x @ W^T), axis=-1) + bias.

Full shapes: x [8192, 4096] f32, weight [4096, 4096] f32, bias [4096] f32.
Sharding: data-parallel over rows of x across 8 NeuronCores (1024 rows/core);
weight and bias replicated. Matmul runs in bf16 on the PE array with fp32
PSUM accumulation; gelu is computed with the exact tanh-approx constants of
the reference via DVE + ACT(Tanh), and softmax needs no max-subtraction
because gelu output is bounded in [-0.17, ~3.5] so exp cannot overflow.

Per-core loop structure (MC=1024 rows):
  split rows into G=2 groups of 512; for each group, stream weight n-tiles
  (512 cols) with the x-group resident in SBUF; accumulate 32 k-matmuls into
  PSUM per (m-tile, n-tile); fuse exp(gelu(v)) into the PSUM->SBUF epilogue
  with per-row sums accumulated by the ACT engine; normalize + bias-add with
  one fused DVE op per tile, then DMA out.
"""

import sys

if "/opt/trn_rl_repo" not in sys.path:
    sys.path.insert(0, "/opt/trn_rl_repo")

import ml_dtypes
import numpy as np

import concourse.bass as bass
import concourse.tile as tile
from concourse import bacc, mybir
from concourse.bass_utils import run_bass_kernel_spmd

P = 128
GELU_A = 0.044715
GELU_C = 0.7978845608

# Full-problem constants (hardcoded; harness calls kernel() with these shapes)
FULL_M, FULL_K, FULL_N = 8192, 4096, 4096
NCORES = 8
MC = FULL_M // NCORES  # rows per core
G = 2                  # row groups per core
NT = 512               # n tile (columns per weight tile / psum)


W_SCALE = 64.0  # fp8 only: weight values ~U(-1/64,1/64) sit at e4m3's min-normal
                # boundary; scale into [-1,1] for the matmul, undo via ACT scale.


def build_nc(MC=MC, K=FULL_K, N=FULL_N, G=G, NT=NT, fp8=False):
    """Emit the per-core Bass program. Each core computes MC rows."""
    KO = K // P            # k subtiles of 128
    MG = MC // G           # rows per m-group
    MT = MG // P           # 128-row m-tiles per group
    NTILES = N // NT
    f32 = mybir.dt.float32
    bf16 = mybir.dt.bfloat16
    in_dt = mybir.dt.float8e4 if fp8 else bf16
    kstep = 2 if fp8 else 1  # DoubleRow contracts 2 k-subtiles per matmul
    inv_scale = 1.0 / W_SCALE if fp8 else 1.0

    nc = bacc.Bacc("TRN2", target_bir_lowering=False, debug=False)
    xt = nc.dram_tensor("xt", [G, P, KO, MG], in_dt, kind="ExternalInput").ap()
    wt = nc.dram_tensor("wt", [NTILES, P, KO, NT], in_dt, kind="ExternalInput").ap()
    bias = nc.dram_tensor("bias", [P, N], f32, kind="ExternalInput").ap()
    out = nc.dram_tensor("out", [P, MC // P, N], f32, kind="ExternalOutput").ap()

    with tile.TileContext(nc) as tc:
        # k-chunking of the streaming DMAs: matmuls can start as soon as the
        # first chunk lands (Tile tracks slice-level deps), instead of waiting
        # for a full 4MB tile. x gets one spare slot so the next group's first
        # chunk prefetches while the current group is still computing.
        XCH = 8 if KO % 8 == 0 else 4   # x chunks per group
        KX = KO // XCH
        WCH = 8 if KO % 8 == 0 else 4   # w chunks per n-tile
        KW = KO // WCH
        with (
            tc.tile_pool(name="const", bufs=1) as const_pool,
            tc.tile_pool(name="x", bufs=XCH + 1) as x_pool,
            tc.tile_pool(name="w", bufs=2) as w_pool,
            tc.tile_pool(name="probs", bufs=1) as probs_pool,
            tc.tile_pool(name="tmp", bufs=2) as tmp_pool,
            tc.tile_pool(name="stat", bufs=2) as stat_pool,
            tc.tile_pool(name="stage", bufs=4) as stage_pool,
            tc.tile_pool(name="psum", bufs=8, space="PSUM") as psum_pool,
        ):
            bias_t = const_pool.tile([P, N], f32)
            nc.gpsimd.dma_start(bias_t[:], bias[:])

            for g in range(G):
                xcs = []
                for c in range(XCH):
                    xc = x_pool.tile([P, KX, MG], in_dt, tag="xc")
                    nc.gpsimd.dma_start(xc[:], xt[g, :, c * KX : (c + 1) * KX, :])
                    xcs.append(xc)
                probs = probs_pool.tile([P, MT, N], bf16)
                sums = stat_pool.tile([P, MT * NTILES], f32, tag="sums")
                for j in range(NTILES):
                    w_t = w_pool.tile([P, KO, NT], in_dt)
                    for c in range(WCH):
                        nc.gpsimd.dma_start(
                            w_t[:, c * KW : (c + 1) * KW, :],
                            wt[j, :, c * KW : (c + 1) * KW, :],
                        )
                    for i in range(MT):
                        ps = psum_pool.tile([P, NT], f32)
                        for k in range(0, KO, kstep):
                            if kstep == 2:
                                kc, kl = k // KX, k % KX
                                nc.tensor.matmul(
                                    ps[:],
                                    xcs[kc][:, kl : kl + 2, i * P : (i + 1) * P],
                                    w_t[:, k : k + 2, :],
                                    start=(k == 0),
                                    stop=(k == KO - 2),
                                    perf_mode=mybir.MatmulPerfMode.DoubleRow,
                                )
                            else:
                                nc.tensor.matmul(
                                    ps[:],
                                    xcs[k // KX][:, k % KX, i * P : (i + 1) * P],
                                    w_t[:, k, :],
                                    start=(k == 0),
                                    stop=(k == KO - 1),
                                )
                        # p = exp(gelu(v)) with gelu = 0.5*v*(1+tanh(C*(v+A*v^3)))
                        # (HW: only one PSUM input per instruction -> copy out once)
                        vs = tmp_pool.tile([P, NT], f32, tag="vs")
                        nc.scalar.activation(
                            vs[:], ps[:], mybir.ActivationFunctionType.Copy,
                            bias=0.0, scale=inv_scale,
                        )
                        v2 = tmp_pool.tile([P, NT], f32, tag="v2")
                        nc.vector.tensor_mul(v2[:], vs[:], vs[:])
                        t1 = tmp_pool.tile([P, NT], f32, tag="t1")
                        nc.vector.tensor_scalar(
                            t1[:], v2[:], GELU_A, 1.0,
                            mybir.AluOpType.mult, mybir.AluOpType.add,
                        )
                        t2 = tmp_pool.tile([P, NT], f32, tag="t2")
                        nc.vector.tensor_mul(t2[:], vs[:], t1[:])
                        th = tmp_pool.tile([P, NT], f32, tag="th")
                        nc.scalar.activation(
                            th[:], t2[:], mybir.ActivationFunctionType.Tanh,
                            bias=0.0, scale=GELU_C,
                        )
                        g2 = tmp_pool.tile([P, NT], f32, tag="g2")
                        nc.vector.scalar_tensor_tensor(
                            g2[:], th[:], 1.0, vs[:],
                            mybir.AluOpType.add, mybir.AluOpType.mult,
                        )
                        sidx = i * NTILES + j
                        nc.scalar.activation(
                            probs[:, i, j * NT : (j + 1) * NT], g2[:],
                            mybir.ActivationFunctionType.Exp,
                            bias=0.0, scale=0.5,
                            accum_out=sums[:, sidx : sidx + 1],
                        )
                ssum = stat_pool.tile([P, MT], f32, tag="ssum")
                recips = stat_pool.tile([P, MT], f32, tag="recips")
                for i in range(MT):
                    nc.vector.reduce_sum(
                        ssum[:, i : i + 1],
                        sums[:, i * NTILES : (i + 1) * NTILES],
                        axis=mybir.AxisListType.X,
                    )
                    nc.vector.reciprocal(recips[:, i : i + 1], ssum[:, i : i + 1])
                    for j in range(NTILES):
                        st = stage_pool.tile([P, NT], f32)
                        nc.vector.scalar_tensor_tensor(
                            st[:],
                            probs[:, i, j * NT : (j + 1) * NT],
                            recips[:, i : i + 1],
                            bias_t[:, j * NT : (j + 1) * NT],
                            mybir.AluOpType.mult,
                            mybir.AluOpType.add,
                        )
                        nc.gpsimd.dma_start(out[:, g * MT + i, j * NT : (j + 1) * NT], st[:])
    nc.compile()
    return nc


def pack_inputs(x, weight, bias, MC=MC, G=G, NT=NT, fp8=False):
    """Host-side shard + pack into the DMA-friendly layouts the kernel expects."""
    M, K = x.shape
    N = weight.shape[0]
    KO = K // P
    MG = MC // G
    NTILES = N // NT
    ncores = M // MC
    in_np = mybir.dt.np(mybir.dt.float8e4) if fp8 else ml_dtypes.bfloat16
    w_src = weight * W_SCALE if fp8 else weight
    # wt[j, p, ko, n] = weight[j*NT+n, ko*P+p]
    wt = np.ascontiguousarray(
        w_src.astype(in_np).reshape(NTILES, NT, KO, P).transpose(0, 3, 2, 1)
    )
    bias_b = np.ascontiguousarray(
        np.broadcast_to(bias.astype(np.float32)[None, :], (P, N))
    )
    in_maps = []
    for c in range(ncores):
        xs = x[c * MC : (c + 1) * MC].astype(in_np)
        # xt[g, p, ko, m] = x_core[g*MG+m, ko*P+p]
        xtc = np.ascontiguousarray(xs.reshape(G, MG, KO, P).transpose(0, 3, 2, 1))
        in_maps.append({"xt": xtc, "wt": wt, "bias": bias_b})
    return in_maps


def unpack_outputs(results, MC=MC, N=FULL_N):
    outs = []
    for res in results:
        o = np.asarray(res["out"])  # [P, MC//P, N]
        outs.append(o.transpose(1, 0, 2).reshape(MC, N))
    return np.concatenate(outs, axis=0)


USE_FP8 = False

_CACHE = {}


def _get_nc(fp8=USE_FP8):
    key = ("nc", fp8)
    if key not in _CACHE:
        _CACHE[key] = build_nc(fp8=fp8)
    return _CACHE[key]


def _ensure_trace_env():
    """The agent image's antenv lacks axon_hooks, so NTFF tracing silently
    degrades. Register the ctypes-based hook ourselves, and neuter the S3
    artifact upload (no bucket access here)."""
    try:
        from antenv.axon_hooks import get_axon_ntff_profile_hook  # noqa: F401
    except ImportError:
        import types

        import antenv
        from trn_agent_boot.trn_boot import _ntff_profile_via_ctypes

        mod = types.ModuleType("antenv.axon_hooks")
        state = {"hook": _ntff_profile_via_ctypes("/opt/axon/libaxon_pjrt.so")}
        mod.set_axon_ntff_profile_hook = lambda h: state.__setitem__("hook", h)
        mod.get_axon_ntff_profile_hook = lambda: state["hook"]
        sys.modules["antenv.axon_hooks"] = mod
        antenv.axon_hooks = mod
    import concourse.bass_utils as bu

    bu.upload_artifacts = lambda tmpdir: f"local://{tmpdir}"


def kernel(x, weight, bias, trace=False, fp8=USE_FP8):
    if trace:
        _ensure_trace_env()
    nc = _get_nc(fp8)
    in_maps = pack_inputs(
        np.asarray(x, dtype=np.float32),
        np.asarray(weight, dtype=np.float32),
        np.asarray(bias, dtype=np.float32),
        fp8=fp8,
    )
    res = run_bass_kernel_spmd(nc, in_maps, core_ids=list(range(NCORES)), trace=trace)
    out = unpack_outputs(res.results)
    if trace:
        return out, res
    return out
